# revision 25
# baseline (speedup 1.0000x reference)
"""Bass/Tile TRN2 kernel for a non-local attention block (BaseNonLocalBlock).

Contract: kernel(**inputs) takes the FULL inputs of the nn.Module problem
(B=1, D=256, H=4, N=4096) and returns the FULL output [1, 256, 4096].

Sharding: query columns of the N x N attention are split across the 8
NeuronCores (512 queries per core). K/V projections are computed
redundantly on every core (cheap); each core produces its own output
column slice and the host concatenates.

Per-core algorithm (flash-attention style, scores never hit HBM):
  Q = (Wq/8) @ xq + bq/8              [256, 512]   (1/sqrt(DH) folded in)
  K = Wk @ x + bk                     [256, 4096]
  V_T = x^T @ Wv^T (+ones col/head)   [4096, 4*65] (denominator trick)
  phase 1: project all of K, V_T (PE-dense, overlaps the input DMA ramp)
  phase 2: per key-chunk i (32 x 128 keys), per head-pair:
    S_T[j] = K_h[:, i]^T @ Q_h        [128, 2, 512]  (PSUM, 2 banks)
    E = exp(spatialT[i] * S_T)        one DVE mult + one ACT exp per pair
    msg_h += V_T_aug[i, h]^T @ E[j]   [65, 512]  (PSUM accum; row 64 = denom)
    (message matmuls run one iteration behind so the PE never head-of-line
    blocks on the DVE->ACT chain; spt tiles prefetched on the gpsimd ring)
  msg = msg_h[0:64] / msg_h[64]  (gpsimd partition_broadcast + DVE recip)
  out = xq + W3 @ relu(bn2(W2 @ relu(bn1(W1 @ msg))))   (BN folded into W/b)

Matmul operands are bf16 (fp32/f32r pay a serialized two-pass weight load
on the PE); accumulation stays fp32 in PSUM, and the residual add reads a
separate fp32 copy of x so the dominant term is exact.
"""

import numpy as np
from contextlib import ExitStack

D = 256
N = 4096
NQ = 512          # queries per core
H = 4
DH = 64
NCORES = 8
NIT = N // 128    # 32 key chunks
VTS = 68          # padded per-head stride in the V_T-aug tile

_CACHE = {}


def _build(has_bq, has_bk, has_bv, has_b3):
    import concourse.bass as bass
    import concourse.tile as tile
    from concourse import bacc, mybir

    F32 = mybir.dt.float32
    BF16 = mybir.dt.bfloat16
    Id = mybir.ActivationFunctionType.Identity
    Exp = mybir.ActivationFunctionType.Exp
    Relu = mybir.ActivationFunctionType.Relu

    nc = bacc.Bacc("TRN2", target_bir_lowering=False, debug=False,
                   num_devices=NCORES)

    # DRAM I/O (per core)
    x_d = nc.dram_tensor("x", [D, N], BF16, kind="ExternalInput").ap()
    xq_d = nc.dram_tensor("xq", [D, NQ], BF16, kind="ExternalInput").ap()
    xqr_d = nc.dram_tensor("xqr", [D, NQ], F32, kind="ExternalInput").ap()
    spt_d = nc.dram_tensor("spt", [N, NQ], BF16, kind="ExternalInput").ap()
    wqt_d = nc.dram_tensor("wqt", [D, D], BF16, kind="ExternalInput").ap()
    wkt_d = nc.dram_tensor("wkt", [D, D], BF16, kind="ExternalInput").ap()
    wvt_d = nc.dram_tensor("wvt", [D, D], BF16, kind="ExternalInput").ap()
    w1t_d = nc.dram_tensor("w1t", [D, 128], BF16, kind="ExternalInput").ap()
    w2t_d = nc.dram_tensor("w2t", [128, 128], BF16, kind="ExternalInput").ap()
    w3t_d = nc.dram_tensor("w3t", [128, D], BF16, kind="ExternalInput").ap()
    bq_d = nc.dram_tensor("bq2", [128, 2], F32, kind="ExternalInput").ap()
    bk_d = nc.dram_tensor("bk2", [128, 2], F32, kind="ExternalInput").ap()
    bv_d = nc.dram_tensor("bv2", [128, 2], F32, kind="ExternalInput").ap()
    b1_d = nc.dram_tensor("b1f", [128, 1], F32, kind="ExternalInput").ap()
    b2_d = nc.dram_tensor("b2f", [128, 1], F32, kind="ExternalInput").ap()
    b3_d = nc.dram_tensor("b32", [128, 2], F32, kind="ExternalInput").ap()
    out_d = nc.dram_tensor("out", [D, NQ], F32, kind="ExternalOutput").ap()

    spt_t3 = spt_d.rearrange("(t p) o -> t p o", p=128)

    with tile.TileContext(nc) as tc, ExitStack() as ctx:
        sb = ctx.enter_context(tc.tile_pool(name="sb", bufs=1))
        spt_pool = ctx.enter_context(tc.tile_pool(name="sptp", bufs=8))
        e_pool = ctx.enter_context(tc.tile_pool(name="ep", bufs=5))
        pj_ctx = ExitStack()
        pj = pj_ctx.enter_context(tc.tile_pool(name="pj", bufs=3, space="PSUM"))

        # ---- weights + Q inputs first: Q/K/V projections unblock early ----
        wqt = [sb.tile([128, D], BF16, name=f"wqt{ci}") for ci in range(2)]
        wkt = [sb.tile([128, D], BF16, name=f"wkt{ci}") for ci in range(2)]
        wvt = [sb.tile([128, D], BF16, name=f"wvt{ci}") for ci in range(2)]
        # x as 2 (row-chunk) x 8 (column-block) tiles for fine-grained deps
        xcb = [[sb.tile([128, 512], BF16, name=f"x{ci}_{ib}") for ib in range(8)]
               for ci in range(2)]
        xq = [sb.tile([128, NQ], BF16, name=f"xq{co}") for co in range(2)]
        bq = sb.tile([128, 2], F32, name="bq")
        bk = sb.tile([128, 2], F32, name="bk")

        def load_x(ib):
            for ci in range(2):
                nc.sync.dma_start(xcb[ci][ib][:],
                                  x_d[ci * 128:(ci + 1) * 128,
                                      ib * 512:(ib + 1) * 512])

        # issue order tuned so K-proj(ib0) inputs land right as the PE
        # warmup ends (~10.5us): wkt + x(ib0) first, everything else after
        for ci in range(2):
            nc.sync.dma_start(wkt[ci][:], wkt_d[ci * 128:(ci + 1) * 128, :])
        load_x(0)
        for ci in range(2):
            nc.sync.dma_start(wvt[ci][:], wvt_d[ci * 128:(ci + 1) * 128, :])
        load_x(1)
        for ci in range(2):
            nc.sync.dma_start(wqt[ci][:], wqt_d[ci * 128:(ci + 1) * 128, :])
        for co in range(2):
            nc.sync.dma_start(xq[co][:], xq_d[co * 128:(co + 1) * 128, :])
        nc.sync.dma_start(bq[:], bq_d[:, :])
        nc.sync.dma_start(bk[:], bk_d[:, :])
        for ib in range(2, 8):
            load_x(ib)

        k_sb = [sb.tile([128, N], BF16, name=f"k{co}") for co in range(2)]
        q_sb = [sb.tile([128, NQ], BF16, name=f"q{co}") for co in range(2)]
        # V^T augmented: per key-chunk it, per head h: [64 V cols | ones | pad]
        vt = sb.tile([128, NIT, H, VTS], BF16, name="vt")
        nc.gpsimd.memset(vt[:, :, :, 64:65], 1.0)
        msg = [sb.tile([128, NQ], BF16, name=f"msg{co}") for co in range(2)]

        # ---- PE warmup: ~4us of tiny matmuls so HAM unthrottles during the
        # DMA ramp (dummy operands; result never read) ----
        warm = sb.tile([128, 64], BF16, name="warm")
        nc.vector.memset(warm[:].bitcast(F32)[:, 0:32], 0.0)
        wps = pj.tile([128, 2, NQ], F32, tag="t")
        for r in range(32):
            nc.tensor.matmul(wps[0:64, 0, 0:64], warm[:], warm[:],
                             start=True, stop=True)


        # ---- main streaming loop over key chunks ----
        # spt prefetch on the (otherwise idle) GPSIMD DMA ring
        spt_tiles = {}

        def load_spt(it):
            t = spt_pool.tile([128, NQ], BF16, tag="spt")
            nc.gpsimd.dma_start(t[:], spt_t3[it])
            spt_tiles[it] = t

        for it in range(4):
            load_spt(it)

        # message matmuls run one iteration behind the scores/mask/exp chain
        # so the PE never waits on the DVE->ACT pipeline mid-iteration
        pend = None

        def emit_msg(p, hp):
            pit, e2s = p
            for j in range(2):
                h = 2 * hp + j
                nc.tensor.matmul(mps[h][:], vt[:, pit, h, 0:65],
                                 e2s[hp][:, j, :],
                                 start=(pit == 0), stop=(pit == NIT - 1))

        # ---- projection phase: all K and V^T blocks (overlaps the DMA ramp,
        # keeps the PE dense/warm; leaves the attention loop contention-free)
        cp = [0]

        def emit_qproj():
            for co in range(2):
                ps = pj.tile([128, NQ], F32, tag="t")
                for ci in range(2):
                    nc.tensor.matmul(ps[:],
                                     wqt[ci][:, co * 128:(co + 1) * 128],
                                     xq[ci][:],
                                     start=(ci == 0), stop=(ci == 1))
                if has_bq:
                    nc.scalar.activation(q_sb[co][:], ps[:], Id,
                                         bias=bq[:, co:co + 1])
                else:
                    nc.scalar.copy(q_sb[co][:], ps[:])

        for ib in range(8):
            if ib == 1:
                emit_qproj()
            for co in range(2):
                ps = pj.tile([128, 2, NQ], F32, tag="t")
                for ci in range(2):
                    nc.tensor.matmul(ps[:, 0, :],
                                     wkt[ci][:, co * 128:(co + 1) * 128],
                                     xcb[ci][ib][:],
                                     start=(ci == 0), stop=(ci == 1))
                ksl = k_sb[co][:, ib * 512:(ib + 1) * 512]
                if has_bk:
                    nc.scalar.activation(ksl, ps[:, 0, :], Id,
                                         bias=bk[:, co:co + 1])
                elif cp[0] % 2 == 0:
                    nc.scalar.copy(ksl, ps[:, 0, :])
                else:
                    nc.vector.tensor_copy(ksl, ps[:, 0, :])
                cp[0] += 1
            for itp in range(ib * 4, ib * 4 + 4, 2):
                vps = pj.tile([128, 2, NQ], F32, tag="t")
                for w in range(2):
                    icol = slice(((itp + w) % 4) * 128,
                                 ((itp + w) % 4) * 128 + 128)
                    for ci in range(2):
                        nc.tensor.matmul(vps[:, w, 0:D],
                                         xcb[ci][ib][:, icol],
                                         wvt[ci][:],
                                         start=(ci == 0), stop=(ci == 1))
                vdst = vt[:, itp:itp + 2, :, 0:64]
                vsrc = vps[:, 0:2, 0:D].rearrange("p w (h c) -> p w h c", h=H)
                if cp[0] % 2 == 0:
                    nc.scalar.copy(vdst, vsrc)
                else:
                    nc.vector.tensor_copy(vdst, vsrc)
                cp[0] += 1

        pj_ctx.close()
        ps_t = ctx.enter_context(tc.tile_pool(name="pst", bufs=2, space="PSUM"))
        ps_m = ctx.enter_context(tc.tile_pool(name="psm", bufs=1, space="PSUM"))
        mps = [ps_m.tile([65, NQ], F32, name=f"mps{h}") for h in range(H)]

        # ---- attention loop: pure scores -> mask-mult -> exp -> message ----
        for it in range(NIT):
            if True:
                if it + 4 < NIT:
                    load_spt(it + 4)
                spt_t = spt_tiles.pop(it)
                # broadcast the mask over the head pair (free-dim 0-stride)
                spt_b = bass.AP(tensor=spt_t.tensor, offset=spt_t.offset,
                                ap=[list(spt_t.ap[0]), [0, 2],
                                    list(spt_t.ap[1])])
                e2s = []
                for hp in range(2):
                    sps = ps_t.tile([128, 2, NQ], F32, tag="t")
                    for j in range(2):
                        ro = j * 64
                        nc.tensor.matmul(
                            sps[:, j, :],
                            k_sb[hp][ro:ro + 64, it * 128:(it + 1) * 128],
                            q_sb[hp][ro:ro + 64, :],
                            start=True, stop=True)
                    el = e_pool.tile([128, 2, NQ], BF16, tag="el")
                    nc.vector.tensor_mul(el[:], sps[:], spt_b)
                    e2 = e_pool.tile([128, 2, NQ], BF16, tag="e")
                    nc.scalar.activation(e2[:], el[:], Exp)
                    e2s.append(e2)
                    if pend is not None:
                        emit_msg(pend, hp)
                pend = (it, e2s)
        emit_msg(pend, 0)
        emit_msg(pend, 1)

        # ---- late inputs (only needed after the attention loop) ----
        w1t = [sb.tile([128, 128], BF16, name=f"w1t{ci}") for ci in range(2)]
        for ci in range(2):
            nc.sync.dma_start(w1t[ci][:], w1t_d[ci * 128:(ci + 1) * 128, :])
        w2t = sb.tile([128, 128], BF16, name="w2t")
        nc.sync.dma_start(w2t[:], w2t_d[:, :])
        w3t = sb.tile([128, D], BF16, name="w3t")
        nc.sync.dma_start(w3t[:], w3t_d[:, :])
        xqr = [sb.tile([128, NQ], F32, name=f"xqr{co}") for co in range(2)]
        for co in range(2):
            nc.sync.dma_start(xqr[co][:], xqr_d[co * 128:(co + 1) * 128, :])
        b1 = sb.tile([128, 1], F32, name="b1")
        b2 = sb.tile([128, 1], F32, name="b2")
        nc.sync.dma_start(b1[:], b1_d[:, :])
        nc.sync.dma_start(b2[:], b2_d[:, :])
        if has_bv:
            bv = sb.tile([128, 2], F32, name="bv")
            nc.sync.dma_start(bv[:], bv_d[:, :])
        if has_b3:
            b3 = sb.tile([128, 2], F32, name="b3")
            nc.sync.dma_start(b3[:], b3_d[:, :])

        # ---- softmax normalization ----
        scr = sb.tile([64, NQ], F32, name="scr")
        for h in range(H):
            co, ro = h // 2, (h % 2) * 64
            dh = sb.tile([1, NQ], F32, name=f"dh{h}")
            nc.scalar.copy(dh[:], mps[h][64:65, :])
            dbc = sb.tile([64, NQ], F32, name=f"dbc{h}")
            nc.gpsimd.partition_broadcast(dbc[:], dh[:], channels=64)
            rbc = sb.tile([64, NQ], F32, name=f"rbc{h}")
            nc.vector.reciprocal_approx_accurate(out=rbc[:], in_=dbc[:],
                                                 scratch=scr[:])
            nc.vector.tensor_mul(msg[co][ro:ro + 64, :], mps[h][0:64, :], rbc[:])
            if has_bv:
                nc.scalar.activation(msg[co][ro:ro + 64, :],
                                     msg[co][ro:ro + 64, :], Id,
                                     bias=bv[ro:ro + 64, co:co + 1])

        # ---- message MLP + residual ----
        u1 = ps_t.tile([128, 2, NQ], F32, tag="t")
        for ci in range(2):
            nc.tensor.matmul(u1[:, 0, :], w1t[ci][:], msg[ci][:],
                             start=(ci == 0), stop=(ci == 1))
        h1 = sb.tile([128, NQ], BF16, name="h1")
        nc.scalar.activation(h1[:], u1[:, 0, :], Relu, bias=b1[:, 0:1])
        u2 = ps_t.tile([128, 2, NQ], F32, tag="t")
        nc.tensor.matmul(u2[:, 0, :], w2t[:], h1[:], start=True, stop=True)
        h2 = sb.tile([128, NQ], BF16, name="h2")
        nc.scalar.activation(h2[:], u2[:, 0, :], Relu, bias=b2[:, 0:1])
        for co in range(2):
            u3 = ps_t.tile([128, 2, NQ], F32, tag="t")
            nc.tensor.matmul(u3[:, 0, :], w3t[:, co * 128:(co + 1) * 128],
                             h2[:], start=True, stop=True)
            ot = sb.tile([128, NQ], F32, name=f"ot{co}")
            if has_b3:
                tb = sb.tile([128, NQ], F32, name=f"tb{co}")
                nc.scalar.activation(tb[:], u3[:, 0, :], Id, bias=b3[:, co:co + 1])
                nc.vector.tensor_add(ot[:], tb[:], xqr[co][:])
            else:
                nc.vector.tensor_add(ot[:], u3[:, 0, :], xqr[co][:])
            nc.sync.dma_start(out_d[co * 128:(co + 1) * 128, :], ot[:])

    nc.compile()
    return nc


def _prep_inputs(inputs):
    import ml_dtypes
    bf = lambda a: np.ascontiguousarray(
        np.asarray(a, dtype=np.float32).astype(ml_dtypes.bfloat16))
    f = lambda a: np.ascontiguousarray(np.asarray(a, dtype=np.float32))
    x32 = f(inputs["corr_feat_belief"][0])                  # [D, N]
    spT = np.asarray(inputs["spatial_compatibility"][0]).T  # [N(keys), N(queries)]
    Wq, bq = f(inputs["Wq"]), f(inputs["bq"])
    Wk, bk = f(inputs["Wk"]), f(inputs["bk"])
    Wv, bv = f(inputs["Wv"]), f(inputs["bv"])
    W1, b1, g1, be1 = f(inputs["W1"]), f(inputs["b1"]), f(inputs["g1"]), f(inputs["be1"])
    W2, b2, g2, be2 = f(inputs["W2"]), f(inputs["b2"]), f(inputs["g2"]), f(inputs["be2"])
    W3, b3 = f(inputs["W3"]), f(inputs["b3"])

    scale = np.float32(1.0 / np.sqrt(DH))
    s1 = (g1 / np.sqrt(np.float32(1.0) + np.float32(1e-5))).astype(np.float32)
    s2 = (g2 / np.sqrt(np.float32(1.0) + np.float32(1e-5))).astype(np.float32)

    spT_bf = bf(spT)
    x_bf = bf(x32)
    common = dict(
        x=x_bf,
        wqt=bf(Wq.T * scale),
        wkt=bf(Wk.T),
        wvt=bf(Wv.T),
        w1t=bf((W1 * s1[:, None]).T),
        w2t=bf((W2 * s2[:, None]).T),
        w3t=bf(W3.T),
        bq2=f((bq * scale).reshape(2, 128).T),
        bk2=f(bk.reshape(2, 128).T),
        bv2=f(bv.reshape(2, 128).T),
        b1f=f((s1 * b1 + be1).reshape(128, 1)),
        b2f=f((s2 * b2 + be2).reshape(128, 1)),
        b32=f(b3.reshape(2, 128).T),
    )
    in_maps = []
    for m in range(NCORES):
        sl = slice(m * NQ, (m + 1) * NQ)
        im = dict(common)
        im["xq"] = np.ascontiguousarray(x_bf[:, sl])
        im["xqr"] = f(x32[:, sl])
        im["spt"] = np.ascontiguousarray(spT_bf[:, sl])
        in_maps.append(im)
    flags = tuple(bool(np.any(b != 0)) for b in (bq, bk, bv, b3))
    return in_maps, flags


def _run(inputs, trace=False):
    from concourse.bass_utils import run_bass_kernel_spmd
    in_maps, flags = _prep_inputs(inputs)
    if flags not in _CACHE:
        _CACHE[flags] = _build(*flags)
    nc = _CACHE[flags]
    res = run_bass_kernel_spmd(nc, in_maps, core_ids=list(range(NCORES)),
                               trace=trace)
    out = np.concatenate([res.results[m]["out"] for m in range(NCORES)],
                         axis=1)[None]
    return np.ascontiguousarray(out.astype(np.float32)), res


def kernel(**inputs):
    out, _ = _run(inputs, trace=False)
    return out


# revision 26
# speedup vs baseline: 1.0072x; 1.0072x over previous
"""Bass/Tile TRN2 kernel for a non-local attention block (BaseNonLocalBlock).

Contract: kernel(**inputs) takes the FULL inputs of the nn.Module problem
(B=1, D=256, H=4, N=4096) and returns the FULL output [1, 256, 4096].

Sharding: query columns of the N x N attention are split across the 8
NeuronCores (512 queries per core). K/V projections are computed
redundantly on every core (cheap); each core produces its own output
column slice and the host concatenates.

Per-core algorithm (flash-attention style, scores never hit HBM):
  Q = (Wq/8) @ xq + bq/8              [256, 512]   (1/sqrt(DH) folded in)
  K = Wk @ x + bk                     [256, 4096]
  V_T = x^T @ Wv^T (+ones col/head)   [4096, 4*65] (denominator trick)
  phase 1: project all of K, V_T (PE-dense, overlaps the input DMA ramp)
  phase 2: per key-chunk i (32 x 128 keys), per head-pair:
    S_T[j] = K_h[:, i]^T @ Q_h        [128, 2, 512]  (PSUM, 2 banks)
    E = exp(spatialT[i] * S_T)        one DVE mult + one ACT exp per pair
    msg_h += V_T_aug[i, h]^T @ E[j]   [65, 512]  (PSUM accum; row 64 = denom)
    (message matmuls run one iteration behind so the PE never head-of-line
    blocks on the DVE->ACT chain; spt tiles prefetched on the gpsimd ring)
  msg = msg_h[0:64] / msg_h[64]  (gpsimd partition_broadcast + DVE recip)
  out = xq + W3 @ relu(bn2(W2 @ relu(bn1(W1 @ msg))))   (BN folded into W/b)

Matmul operands are bf16 (fp32/f32r pay a serialized two-pass weight load
on the PE); accumulation stays fp32 in PSUM, and the residual add reads a
separate fp32 copy of x so the dominant term is exact.
"""

import numpy as np
from contextlib import ExitStack

D = 256
N = 4096
NQ = 512          # queries per core
H = 4
DH = 64
NCORES = 8
NIT = N // 128    # 32 key chunks
VTS = 68          # padded per-head stride in the V_T-aug tile

_CACHE = {}


def _build(has_bq, has_bk, has_bv, has_b3):
    import concourse.bass as bass
    import concourse.tile as tile
    from concourse import bacc, mybir

    F32 = mybir.dt.float32
    BF16 = mybir.dt.bfloat16
    Id = mybir.ActivationFunctionType.Identity
    Exp = mybir.ActivationFunctionType.Exp
    Relu = mybir.ActivationFunctionType.Relu

    nc = bacc.Bacc("TRN2", target_bir_lowering=False, debug=False,
                   num_devices=NCORES)

    # DRAM I/O (per core)
    x_d = nc.dram_tensor("x", [D, N], BF16, kind="ExternalInput").ap()
    xq_d = nc.dram_tensor("xq", [D, NQ], BF16, kind="ExternalInput").ap()
    xqr_d = nc.dram_tensor("xqr", [D, NQ], F32, kind="ExternalInput").ap()
    spt_d = nc.dram_tensor("spt", [N, NQ], BF16, kind="ExternalInput").ap()
    wqt_d = nc.dram_tensor("wqt", [D, D], BF16, kind="ExternalInput").ap()
    wkt_d = nc.dram_tensor("wkt", [D, D], BF16, kind="ExternalInput").ap()
    wvt_d = nc.dram_tensor("wvt", [D, D], BF16, kind="ExternalInput").ap()
    w1t_d = nc.dram_tensor("w1t", [D, 128], BF16, kind="ExternalInput").ap()
    w2t_d = nc.dram_tensor("w2t", [128, 128], BF16, kind="ExternalInput").ap()
    w3t_d = nc.dram_tensor("w3t", [128, D], BF16, kind="ExternalInput").ap()
    bq_d = nc.dram_tensor("bq2", [128, 2], F32, kind="ExternalInput").ap()
    bk_d = nc.dram_tensor("bk2", [128, 2], F32, kind="ExternalInput").ap()
    bv_d = nc.dram_tensor("bv2", [128, 2], F32, kind="ExternalInput").ap()
    b1_d = nc.dram_tensor("b1f", [128, 1], F32, kind="ExternalInput").ap()
    b2_d = nc.dram_tensor("b2f", [128, 1], F32, kind="ExternalInput").ap()
    b3_d = nc.dram_tensor("b32", [128, 2], F32, kind="ExternalInput").ap()
    out_d = nc.dram_tensor("out", [D, NQ], F32, kind="ExternalOutput").ap()

    spt_t3 = spt_d.rearrange("(t p) o -> t p o", p=128)

    with tile.TileContext(nc) as tc, ExitStack() as ctx:
        sb = ctx.enter_context(tc.tile_pool(name="sb", bufs=1))
        spt_pool = ctx.enter_context(tc.tile_pool(name="sptp", bufs=8))
        e_pool = ctx.enter_context(tc.tile_pool(name="ep", bufs=5))
        pj_ctx = ExitStack()
        pj = pj_ctx.enter_context(tc.tile_pool(name="pj", bufs=3, space="PSUM"))

        # ---- weights + Q inputs first: Q/K/V projections unblock early ----
        wqt = [sb.tile([128, D], BF16, name=f"wqt{ci}") for ci in range(2)]
        wkt = [sb.tile([128, D], BF16, name=f"wkt{ci}") for ci in range(2)]
        wvt = [sb.tile([128, D], BF16, name=f"wvt{ci}") for ci in range(2)]
        # x as 2 (row-chunk) x 8 (column-block) tiles for fine-grained deps
        xcb = [[sb.tile([128, 512], BF16, name=f"x{ci}_{ib}") for ib in range(8)]
               for ci in range(2)]
        xq = [sb.tile([128, NQ], BF16, name=f"xq{co}") for co in range(2)]
        bq = sb.tile([128, 2], F32, name="bq")
        bk = sb.tile([128, 2], F32, name="bk")

        def load_x(ib):
            for ci in range(2):
                nc.sync.dma_start(xcb[ci][ib][:],
                                  x_d[ci * 128:(ci + 1) * 128,
                                      ib * 512:(ib + 1) * 512])

        # issue order tuned so K-proj(ib0) inputs land right as the PE
        # warmup ends (~10.5us): wkt + x(ib0) first, everything else after
        for ci in range(2):
            nc.sync.dma_start(wkt[ci][:], wkt_d[ci * 128:(ci + 1) * 128, :])
        load_x(0)
        for ci in range(2):
            nc.sync.dma_start(wqt[ci][:], wqt_d[ci * 128:(ci + 1) * 128, :])
        for co in range(2):
            nc.sync.dma_start(xq[co][:], xq_d[co * 128:(co + 1) * 128, :])
        for ci in range(2):
            nc.sync.dma_start(wvt[ci][:], wvt_d[ci * 128:(ci + 1) * 128, :])
        load_x(1)
        nc.sync.dma_start(bq[:], bq_d[:, :])
        nc.sync.dma_start(bk[:], bk_d[:, :])
        for ib in range(2, 8):
            load_x(ib)

        k_sb = [sb.tile([128, N], BF16, name=f"k{co}") for co in range(2)]
        q_sb = [sb.tile([128, NQ], BF16, name=f"q{co}") for co in range(2)]
        # V^T augmented: per key-chunk it, per head h: [64 V cols | ones | pad]
        vt = sb.tile([128, NIT, H, VTS], BF16, name="vt")
        nc.gpsimd.memset(vt[:, :, :, 64:65], 1.0)
        msg = [sb.tile([128, NQ], BF16, name=f"msg{co}") for co in range(2)]

        # ---- PE warmup: ~4us of tiny matmuls so HAM unthrottles during the
        # DMA ramp (dummy operands; result never read) ----
        warm = sb.tile([128, 64], BF16, name="warm")
        nc.vector.memset(warm[:].bitcast(F32)[:, 0:32], 0.0)
        wps = pj.tile([128, 2, NQ], F32, tag="t")
        for r in range(44):
            nc.tensor.matmul(wps[0:64, 0, 0:64], warm[:], warm[:],
                             start=True, stop=True)


        # ---- main streaming loop over key chunks ----
        # spt prefetch on the (otherwise idle) GPSIMD DMA ring
        spt_tiles = {}

        def load_spt(it):
            t = spt_pool.tile([128, NQ], BF16, tag="spt")
            nc.gpsimd.dma_start(t[:], spt_t3[it])
            spt_tiles[it] = t

        for it in range(4):
            load_spt(it)

        # message matmuls run one iteration behind the scores/mask/exp chain
        # so the PE never waits on the DVE->ACT pipeline mid-iteration
        pend = None

        def emit_msg(p, hp):
            pit, e2s = p
            for j in range(2):
                h = 2 * hp + j
                nc.tensor.matmul(mps[h][:], vt[:, pit, h, 0:65],
                                 e2s[hp][:, j, :],
                                 start=(pit == 0), stop=(pit == NIT - 1))

        # ---- projection phase: all K and V^T blocks (overlaps the DMA ramp,
        # keeps the PE dense/warm; leaves the attention loop contention-free)
        cp = [0]

        def emit_qproj():
            for co in range(2):
                ps = pj.tile([128, NQ], F32, tag="t")
                for ci in range(2):
                    nc.tensor.matmul(ps[:],
                                     wqt[ci][:, co * 128:(co + 1) * 128],
                                     xq[ci][:],
                                     start=(ci == 0), stop=(ci == 1))
                if has_bq:
                    nc.scalar.activation(q_sb[co][:], ps[:], Id,
                                         bias=bq[:, co:co + 1])
                else:
                    nc.scalar.copy(q_sb[co][:], ps[:])

        for ib in range(8):
            if ib == 1:
                emit_qproj()
            for co in range(2):
                ps = pj.tile([128, 2, NQ], F32, tag="t")
                for ci in range(2):
                    nc.tensor.matmul(ps[:, 0, :],
                                     wkt[ci][:, co * 128:(co + 1) * 128],
                                     xcb[ci][ib][:],
                                     start=(ci == 0), stop=(ci == 1))
                ksl = k_sb[co][:, ib * 512:(ib + 1) * 512]
                if has_bk:
                    nc.scalar.activation(ksl, ps[:, 0, :], Id,
                                         bias=bk[:, co:co + 1])
                elif cp[0] % 2 == 0:
                    nc.scalar.copy(ksl, ps[:, 0, :])
                else:
                    nc.vector.tensor_copy(ksl, ps[:, 0, :])
                cp[0] += 1
            for itp in range(ib * 4, ib * 4 + 4, 2):
                vps = pj.tile([128, 2, NQ], F32, tag="t")
                for w in range(2):
                    icol = slice(((itp + w) % 4) * 128,
                                 ((itp + w) % 4) * 128 + 128)
                    for ci in range(2):
                        nc.tensor.matmul(vps[:, w, 0:D],
                                         xcb[ci][ib][:, icol],
                                         wvt[ci][:],
                                         start=(ci == 0), stop=(ci == 1))
                vdst = vt[:, itp:itp + 2, :, 0:64]
                vsrc = vps[:, 0:2, 0:D].rearrange("p w (h c) -> p w h c", h=H)
                if cp[0] % 2 == 0:
                    nc.scalar.copy(vdst, vsrc)
                else:
                    nc.vector.tensor_copy(vdst, vsrc)
                cp[0] += 1

        pj_ctx.close()
        ps_t = ctx.enter_context(tc.tile_pool(name="pst", bufs=2, space="PSUM"))
        ps_m = ctx.enter_context(tc.tile_pool(name="psm", bufs=1, space="PSUM"))
        mps = [ps_m.tile([65, NQ], F32, name=f"mps{h}") for h in range(H)]

        # ---- attention loop: pure scores -> mask-mult -> exp -> message ----
        for it in range(NIT):
            if True:
                if it + 4 < NIT:
                    load_spt(it + 4)
                spt_t = spt_tiles.pop(it)
                # broadcast the mask over the head pair (free-dim 0-stride)
                spt_b = bass.AP(tensor=spt_t.tensor, offset=spt_t.offset,
                                ap=[list(spt_t.ap[0]), [0, 2],
                                    list(spt_t.ap[1])])
                e2s = []
                for hp in range(2):
                    sps = ps_t.tile([128, 2, NQ], F32, tag="t")
                    for j in range(2):
                        ro = j * 64
                        nc.tensor.matmul(
                            sps[:, j, :],
                            k_sb[hp][ro:ro + 64, it * 128:(it + 1) * 128],
                            q_sb[hp][ro:ro + 64, :],
                            start=True, stop=True)
                    el = e_pool.tile([128, 2, NQ], BF16, tag="el")
                    nc.vector.tensor_mul(el[:], sps[:], spt_b)
                    e2 = e_pool.tile([128, 2, NQ], BF16, tag="e")
                    nc.scalar.activation(e2[:], el[:], Exp)
                    e2s.append(e2)
                    if pend is not None:
                        emit_msg(pend, hp)
                pend = (it, e2s)
        emit_msg(pend, 0)
        emit_msg(pend, 1)

        # ---- late inputs (only needed after the attention loop) ----
        w1t = [sb.tile([128, 128], BF16, name=f"w1t{ci}") for ci in range(2)]
        for ci in range(2):
            nc.sync.dma_start(w1t[ci][:], w1t_d[ci * 128:(ci + 1) * 128, :])
        w2t = sb.tile([128, 128], BF16, name="w2t")
        nc.sync.dma_start(w2t[:], w2t_d[:, :])
        w3t = sb.tile([128, D], BF16, name="w3t")
        nc.sync.dma_start(w3t[:], w3t_d[:, :])
        xqr = [sb.tile([128, NQ], F32, name=f"xqr{co}") for co in range(2)]
        for co in range(2):
            nc.sync.dma_start(xqr[co][:], xqr_d[co * 128:(co + 1) * 128, :])
        b1 = sb.tile([128, 1], F32, name="b1")
        b2 = sb.tile([128, 1], F32, name="b2")
        nc.sync.dma_start(b1[:], b1_d[:, :])
        nc.sync.dma_start(b2[:], b2_d[:, :])
        if has_bv:
            bv = sb.tile([128, 2], F32, name="bv")
            nc.sync.dma_start(bv[:], bv_d[:, :])
        if has_b3:
            b3 = sb.tile([128, 2], F32, name="b3")
            nc.sync.dma_start(b3[:], b3_d[:, :])

        # ---- softmax normalization ----
        scr = sb.tile([64, NQ], F32, name="scr")
        for h in range(H):
            co, ro = h // 2, (h % 2) * 64
            dh = sb.tile([1, NQ], F32, name=f"dh{h}")
            nc.scalar.copy(dh[:], mps[h][64:65, :])
            dbc = sb.tile([64, NQ], F32, name=f"dbc{h}")
            nc.gpsimd.partition_broadcast(dbc[:], dh[:], channels=64)
            rbc = sb.tile([64, NQ], F32, name=f"rbc{h}")
            nc.vector.reciprocal_approx_accurate(out=rbc[:], in_=dbc[:],
                                                 scratch=scr[:])
            nc.vector.tensor_mul(msg[co][ro:ro + 64, :], mps[h][0:64, :], rbc[:])
            if has_bv:
                nc.scalar.activation(msg[co][ro:ro + 64, :],
                                     msg[co][ro:ro + 64, :], Id,
                                     bias=bv[ro:ro + 64, co:co + 1])

        # ---- message MLP + residual ----
        u1 = ps_t.tile([128, 2, NQ], F32, tag="t")
        for ci in range(2):
            nc.tensor.matmul(u1[:, 0, :], w1t[ci][:], msg[ci][:],
                             start=(ci == 0), stop=(ci == 1))
        h1 = sb.tile([128, NQ], BF16, name="h1")
        nc.scalar.activation(h1[:], u1[:, 0, :], Relu, bias=b1[:, 0:1])
        u2 = ps_t.tile([128, 2, NQ], F32, tag="t")
        nc.tensor.matmul(u2[:, 0, :], w2t[:], h1[:], start=True, stop=True)
        h2 = sb.tile([128, NQ], BF16, name="h2")
        nc.scalar.activation(h2[:], u2[:, 0, :], Relu, bias=b2[:, 0:1])
        for co in range(2):
            u3 = ps_t.tile([128, 2, NQ], F32, tag="t")
            nc.tensor.matmul(u3[:, 0, :], w3t[:, co * 128:(co + 1) * 128],
                             h2[:], start=True, stop=True)
            ot = sb.tile([128, NQ], F32, name=f"ot{co}")
            if has_b3:
                tb = sb.tile([128, NQ], F32, name=f"tb{co}")
                nc.scalar.activation(tb[:], u3[:, 0, :], Id, bias=b3[:, co:co + 1])
                nc.vector.tensor_add(ot[:], tb[:], xqr[co][:])
            else:
                nc.vector.tensor_add(ot[:], u3[:, 0, :], xqr[co][:])
            nc.sync.dma_start(out_d[co * 128:(co + 1) * 128, :], ot[:])

    nc.compile()
    return nc


def _prep_inputs(inputs):
    import ml_dtypes
    bf = lambda a: np.ascontiguousarray(
        np.asarray(a, dtype=np.float32).astype(ml_dtypes.bfloat16))
    f = lambda a: np.ascontiguousarray(np.asarray(a, dtype=np.float32))
    x32 = f(inputs["corr_feat_belief"][0])                  # [D, N]
    spT = np.asarray(inputs["spatial_compatibility"][0]).T  # [N(keys), N(queries)]
    Wq, bq = f(inputs["Wq"]), f(inputs["bq"])
    Wk, bk = f(inputs["Wk"]), f(inputs["bk"])
    Wv, bv = f(inputs["Wv"]), f(inputs["bv"])
    W1, b1, g1, be1 = f(inputs["W1"]), f(inputs["b1"]), f(inputs["g1"]), f(inputs["be1"])
    W2, b2, g2, be2 = f(inputs["W2"]), f(inputs["b2"]), f(inputs["g2"]), f(inputs["be2"])
    W3, b3 = f(inputs["W3"]), f(inputs["b3"])

    scale = np.float32(1.0 / np.sqrt(DH))
    s1 = (g1 / np.sqrt(np.float32(1.0) + np.float32(1e-5))).astype(np.float32)
    s2 = (g2 / np.sqrt(np.float32(1.0) + np.float32(1e-5))).astype(np.float32)

    spT_bf = bf(spT)
    x_bf = bf(x32)
    common = dict(
        x=x_bf,
        wqt=bf(Wq.T * scale),
        wkt=bf(Wk.T),
        wvt=bf(Wv.T),
        w1t=bf((W1 * s1[:, None]).T),
        w2t=bf((W2 * s2[:, None]).T),
        w3t=bf(W3.T),
        bq2=f((bq * scale).reshape(2, 128).T),
        bk2=f(bk.reshape(2, 128).T),
        bv2=f(bv.reshape(2, 128).T),
        b1f=f((s1 * b1 + be1).reshape(128, 1)),
        b2f=f((s2 * b2 + be2).reshape(128, 1)),
        b32=f(b3.reshape(2, 128).T),
    )
    in_maps = []
    for m in range(NCORES):
        sl = slice(m * NQ, (m + 1) * NQ)
        im = dict(common)
        im["xq"] = np.ascontiguousarray(x_bf[:, sl])
        im["xqr"] = f(x32[:, sl])
        im["spt"] = np.ascontiguousarray(spT_bf[:, sl])
        in_maps.append(im)
    flags = tuple(bool(np.any(b != 0)) for b in (bq, bk, bv, b3))
    return in_maps, flags


def _run(inputs, trace=False):
    from concourse.bass_utils import run_bass_kernel_spmd
    in_maps, flags = _prep_inputs(inputs)
    if flags not in _CACHE:
        _CACHE[flags] = _build(*flags)
    nc = _CACHE[flags]
    res = run_bass_kernel_spmd(nc, in_maps, core_ids=list(range(NCORES)),
                               trace=trace)
    out = np.concatenate([res.results[m]["out"] for m in range(NCORES)],
                         axis=1)[None]
    return np.ascontiguousarray(out.astype(np.float32)), res


def kernel(**inputs):
    out, _ = _run(inputs, trace=False)
    return out


# revision 27
# speedup vs baseline: 1.0142x; 1.0069x over previous
"""Bass/Tile TRN2 kernel for a non-local attention block (BaseNonLocalBlock).

Contract: kernel(**inputs) takes the FULL inputs of the nn.Module problem
(B=1, D=256, H=4, N=4096) and returns the FULL output [1, 256, 4096].

Sharding: query columns of the N x N attention are split across the 8
NeuronCores (512 queries per core). K/V projections are computed
redundantly on every core (cheap); each core produces its own output
column slice and the host concatenates.

Per-core algorithm (flash-attention style, scores never hit HBM):
  Q = (Wq/8) @ xq + bq/8              [256, 512]   (1/sqrt(DH) folded in)
  K = Wk @ x + bk                     [256, 4096]
  V_T = x^T @ Wv^T (+ones col/head)   [4096, 4*65] (denominator trick)
  phase 1: project all of K, V_T (PE-dense, overlaps the input DMA ramp)
  phase 2: per key-chunk i (32 x 128 keys), per head-pair:
    S_T[j] = K_h[:, i]^T @ Q_h        [128, 2, 512]  (PSUM, 2 banks)
    E = exp(spatialT[i] * S_T)        one DVE mult + one ACT exp per pair
    msg_h += V_T_aug[i, h]^T @ E[j]   [65, 512]  (PSUM accum; row 64 = denom)
    (message matmuls run one iteration behind so the PE never head-of-line
    blocks on the DVE->ACT chain; spt tiles prefetched on the gpsimd ring)
  msg = msg_h[0:64] / msg_h[64]  (gpsimd partition_broadcast + DVE recip)
  out = xq + W3 @ relu(bn2(W2 @ relu(bn1(W1 @ msg))))   (BN folded into W/b)

Matmul operands are bf16 (fp32/f32r pay a serialized two-pass weight load
on the PE); accumulation stays fp32 in PSUM, and the residual add reads a
separate fp32 copy of x so the dominant term is exact.
"""

import numpy as np
from contextlib import ExitStack

D = 256
N = 4096
NQ = 512          # queries per core
H = 4
DH = 64
NCORES = 8
NIT = N // 128    # 32 key chunks
VTS = 68          # padded per-head stride in the V_T-aug tile

_CACHE = {}


def _build(has_bq, has_bk, has_bv, has_b3):
    import concourse.bass as bass
    import concourse.tile as tile
    from concourse import bacc, mybir

    F32 = mybir.dt.float32
    BF16 = mybir.dt.bfloat16
    Id = mybir.ActivationFunctionType.Identity
    Exp = mybir.ActivationFunctionType.Exp
    Relu = mybir.ActivationFunctionType.Relu

    nc = bacc.Bacc("TRN2", target_bir_lowering=False, debug=False,
                   num_devices=NCORES)

    # DRAM I/O (per core)
    x_d = nc.dram_tensor("x", [D, N], BF16, kind="ExternalInput").ap()
    xq_d = nc.dram_tensor("xq", [D, NQ], BF16, kind="ExternalInput").ap()
    xqr_d = nc.dram_tensor("xqr", [D, NQ], F32, kind="ExternalInput").ap()
    spt_d = nc.dram_tensor("spt", [N, NQ], BF16, kind="ExternalInput").ap()
    wqt_d = nc.dram_tensor("wqt", [D, D], BF16, kind="ExternalInput").ap()
    wkt_d = nc.dram_tensor("wkt", [D, D], BF16, kind="ExternalInput").ap()
    wvt_d = nc.dram_tensor("wvt", [D, D], BF16, kind="ExternalInput").ap()
    w1t_d = nc.dram_tensor("w1t", [D, 128], BF16, kind="ExternalInput").ap()
    w2t_d = nc.dram_tensor("w2t", [128, 128], BF16, kind="ExternalInput").ap()
    w3t_d = nc.dram_tensor("w3t", [128, D], BF16, kind="ExternalInput").ap()
    bq_d = nc.dram_tensor("bq2", [128, 2], F32, kind="ExternalInput").ap()
    bk_d = nc.dram_tensor("bk2", [128, 2], F32, kind="ExternalInput").ap()
    bv_d = nc.dram_tensor("bv2", [128, 2], F32, kind="ExternalInput").ap()
    b1_d = nc.dram_tensor("b1f", [128, 1], F32, kind="ExternalInput").ap()
    b2_d = nc.dram_tensor("b2f", [128, 1], F32, kind="ExternalInput").ap()
    b3_d = nc.dram_tensor("b32", [128, 2], F32, kind="ExternalInput").ap()
    out_d = nc.dram_tensor("out", [D, NQ], F32, kind="ExternalOutput").ap()

    spt_t3 = spt_d.rearrange("(t p) o -> t p o", p=128)

    with tile.TileContext(nc) as tc, ExitStack() as ctx:
        sb = ctx.enter_context(tc.tile_pool(name="sb", bufs=1))
        spt_pool = ctx.enter_context(tc.tile_pool(name="sptp", bufs=8))
        e_pool = ctx.enter_context(tc.tile_pool(name="ep", bufs=5))
        pj_ctx = ExitStack()
        pj = pj_ctx.enter_context(tc.tile_pool(name="pj", bufs=3, space="PSUM"))

        # ---- weights + Q inputs first: Q/K/V projections unblock early ----
        wqt = [sb.tile([128, D], BF16, name=f"wqt{ci}") for ci in range(2)]
        wkt = [sb.tile([128, D], BF16, name=f"wkt{ci}") for ci in range(2)]
        wvt = [sb.tile([128, D], BF16, name=f"wvt{ci}") for ci in range(2)]
        # x as 2 (row-chunk) x 8 (column-block) tiles for fine-grained deps
        xcb = [[sb.tile([128, 512], BF16, name=f"x{ci}_{ib}") for ib in range(8)]
               for ci in range(2)]
        xq = [sb.tile([128, NQ], BF16, name=f"xq{co}") for co in range(2)]
        bq = sb.tile([128, 2], F32, name="bq")
        bk = sb.tile([128, 2], F32, name="bk")

        for ci in range(2):
            sl = slice(ci * 128, (ci + 1) * 128)
            nc.sync.dma_start(wkt[ci][:], wkt_d[sl, :])
            nc.sync.dma_start(wvt[ci][:], wvt_d[sl, :])
            nc.sync.dma_start(wqt[ci][:], wqt_d[sl, :])
        for co in range(2):
            nc.sync.dma_start(xq[co][:], xq_d[co * 128:(co + 1) * 128, :])
        nc.sync.dma_start(bq[:], bq_d[:, :])
        nc.sync.dma_start(bk[:], bk_d[:, :])
        for ib in range(8):
            for ci in range(2):
                nc.sync.dma_start(xcb[ci][ib][:],
                                  x_d[ci * 128:(ci + 1) * 128,
                                      ib * 512:(ib + 1) * 512])

        k_sb = [sb.tile([128, N], BF16, name=f"k{co}") for co in range(2)]
        q_sb = [sb.tile([128, NQ], BF16, name=f"q{co}") for co in range(2)]
        # V^T augmented: per key-chunk it, per head h: [64 V cols | ones | pad]
        vt = sb.tile([128, NIT, H, VTS], BF16, name="vt")
        nc.gpsimd.memset(vt[:, :, :, 64:65], 1.0)
        msg = [sb.tile([128, NQ], BF16, name=f"msg{co}") for co in range(2)]

        # ---- PE warmup: ~4us of tiny matmuls so HAM unthrottles during the
        # DMA ramp (dummy operands; result never read) ----
        warm = sb.tile([128, 64], BF16, name="warm")
        nc.vector.memset(warm[:].bitcast(F32)[:, 0:32], 0.0)
        wps = pj.tile([128, 2, NQ], F32, tag="t")
        for r in range(32):
            nc.tensor.matmul(wps[0:64, 0, 0:64], warm[:], warm[:],
                             start=True, stop=True)


        # ---- main streaming loop over key chunks ----
        # spt prefetch on the (otherwise idle) GPSIMD DMA ring
        spt_tiles = {}

        def load_spt(it):
            t = spt_pool.tile([128, NQ], BF16, tag="spt")
            nc.gpsimd.dma_start(t[:], spt_t3[it])
            spt_tiles[it] = t

        for it in range(4):
            load_spt(it)

        # message matmuls run one iteration behind the scores/mask/exp chain
        # so the PE never waits on the DVE->ACT pipeline mid-iteration
        pend = None

        def emit_msg(p, hp):
            pit, e2s = p
            for j in range(2):
                h = 2 * hp + j
                nc.tensor.matmul(mps[h][:], vt[:, pit, h, 0:65],
                                 e2s[hp][:, j, :],
                                 start=(pit == 0), stop=(pit == NIT - 1))

        # ---- projection phase: all K and V^T blocks (overlaps the DMA ramp,
        # keeps the PE dense/warm; leaves the attention loop contention-free)
        cp = [0]
        for co in range(2):
            ps = pj.tile([128, NQ], F32, tag="t")
            for ci in range(2):
                nc.tensor.matmul(ps[:],
                                 wqt[ci][:, co * 128:(co + 1) * 128],
                                 xq[ci][:],
                                 start=(ci == 0), stop=(ci == 1))
            if has_bq:
                nc.scalar.activation(q_sb[co][:], ps[:], Id,
                                     bias=bq[:, co:co + 1])
            else:
                nc.scalar.copy(q_sb[co][:], ps[:])
        for r in range(16):
            nc.tensor.matmul(wps[0:64, 1, 0:64], warm[:], warm[:],
                             start=True, stop=True)
        for ib in range(8):
            for co in range(2):
                ps = pj.tile([128, 2, NQ], F32, tag="t")
                for ci in range(2):
                    nc.tensor.matmul(ps[:, 0, :],
                                     wkt[ci][:, co * 128:(co + 1) * 128],
                                     xcb[ci][ib][:],
                                     start=(ci == 0), stop=(ci == 1))
                ksl = k_sb[co][:, ib * 512:(ib + 1) * 512]
                if has_bk:
                    nc.scalar.activation(ksl, ps[:, 0, :], Id,
                                         bias=bk[:, co:co + 1])
                elif cp[0] % 2 == 0:
                    nc.scalar.copy(ksl, ps[:, 0, :])
                else:
                    nc.vector.tensor_copy(ksl, ps[:, 0, :])
                cp[0] += 1
            for itp in range(ib * 4, ib * 4 + 4, 2):
                vps = pj.tile([128, 2, NQ], F32, tag="t")
                for w in range(2):
                    icol = slice(((itp + w) % 4) * 128,
                                 ((itp + w) % 4) * 128 + 128)
                    for ci in range(2):
                        nc.tensor.matmul(vps[:, w, 0:D],
                                         xcb[ci][ib][:, icol],
                                         wvt[ci][:],
                                         start=(ci == 0), stop=(ci == 1))
                vdst = vt[:, itp:itp + 2, :, 0:64]
                vsrc = vps[:, 0:2, 0:D].rearrange("p w (h c) -> p w h c", h=H)
                if cp[0] % 2 == 0:
                    nc.scalar.copy(vdst, vsrc)
                else:
                    nc.vector.tensor_copy(vdst, vsrc)
                cp[0] += 1

        pj_ctx.close()
        ps_t = ctx.enter_context(tc.tile_pool(name="pst", bufs=2, space="PSUM"))
        ps_m = ctx.enter_context(tc.tile_pool(name="psm", bufs=1, space="PSUM"))
        mps = [ps_m.tile([65, NQ], F32, name=f"mps{h}") for h in range(H)]

        # ---- attention loop: pure scores -> mask-mult -> exp -> message ----
        for it in range(NIT):
            if True:
                if it + 4 < NIT:
                    load_spt(it + 4)
                spt_t = spt_tiles.pop(it)
                # broadcast the mask over the head pair (free-dim 0-stride)
                spt_b = bass.AP(tensor=spt_t.tensor, offset=spt_t.offset,
                                ap=[list(spt_t.ap[0]), [0, 2],
                                    list(spt_t.ap[1])])
                e2s = []
                for hp in range(2):
                    sps = ps_t.tile([128, 2, NQ], F32, tag="t")
                    for j in range(2):
                        ro = j * 64
                        nc.tensor.matmul(
                            sps[:, j, :],
                            k_sb[hp][ro:ro + 64, it * 128:(it + 1) * 128],
                            q_sb[hp][ro:ro + 64, :],
                            start=True, stop=True)
                    el = e_pool.tile([128, 2, NQ], BF16, tag="el")
                    nc.vector.tensor_mul(el[:], sps[:], spt_b)
                    e2 = e_pool.tile([128, 2, NQ], BF16, tag="e")
                    nc.scalar.activation(e2[:], el[:], Exp)
                    e2s.append(e2)
                    if pend is not None:
                        emit_msg(pend, hp)
                pend = (it, e2s)
        emit_msg(pend, 0)
        emit_msg(pend, 1)

        # ---- late inputs (only needed after the attention loop) ----
        w1t = [sb.tile([128, 128], BF16, name=f"w1t{ci}") for ci in range(2)]
        for ci in range(2):
            nc.sync.dma_start(w1t[ci][:], w1t_d[ci * 128:(ci + 1) * 128, :])
        w2t = sb.tile([128, 128], BF16, name="w2t")
        nc.sync.dma_start(w2t[:], w2t_d[:, :])
        w3t = sb.tile([128, D], BF16, name="w3t")
        nc.sync.dma_start(w3t[:], w3t_d[:, :])
        xqr = [sb.tile([128, NQ], F32, name=f"xqr{co}") for co in range(2)]
        for co in range(2):
            nc.sync.dma_start(xqr[co][:], xqr_d[co * 128:(co + 1) * 128, :])
        b1 = sb.tile([128, 1], F32, name="b1")
        b2 = sb.tile([128, 1], F32, name="b2")
        nc.sync.dma_start(b1[:], b1_d[:, :])
        nc.sync.dma_start(b2[:], b2_d[:, :])
        if has_bv:
            bv = sb.tile([128, 2], F32, name="bv")
            nc.sync.dma_start(bv[:], bv_d[:, :])
        if has_b3:
            b3 = sb.tile([128, 2], F32, name="b3")
            nc.sync.dma_start(b3[:], b3_d[:, :])

        # ---- softmax normalization ----
        scr = sb.tile([64, NQ], F32, name="scr")
        for h in range(H):
            co, ro = h // 2, (h % 2) * 64
            dh = sb.tile([1, NQ], F32, name=f"dh{h}")
            nc.scalar.copy(dh[:], mps[h][64:65, :])
            dbc = sb.tile([64, NQ], F32, name=f"dbc{h}")
            nc.gpsimd.partition_broadcast(dbc[:], dh[:], channels=64)
            rbc = sb.tile([64, NQ], F32, name=f"rbc{h}")
            nc.vector.reciprocal_approx_accurate(out=rbc[:], in_=dbc[:],
                                                 scratch=scr[:])
            nc.vector.tensor_mul(msg[co][ro:ro + 64, :], mps[h][0:64, :], rbc[:])
            if has_bv:
                nc.scalar.activation(msg[co][ro:ro + 64, :],
                                     msg[co][ro:ro + 64, :], Id,
                                     bias=bv[ro:ro + 64, co:co + 1])

        # ---- message MLP + residual ----
        u1 = ps_t.tile([128, 2, NQ], F32, tag="t")
        for ci in range(2):
            nc.tensor.matmul(u1[:, 0, :], w1t[ci][:], msg[ci][:],
                             start=(ci == 0), stop=(ci == 1))
        h1 = sb.tile([128, NQ], BF16, name="h1")
        nc.scalar.activation(h1[:], u1[:, 0, :], Relu, bias=b1[:, 0:1])
        u2 = ps_t.tile([128, 2, NQ], F32, tag="t")
        nc.tensor.matmul(u2[:, 0, :], w2t[:], h1[:], start=True, stop=True)
        h2 = sb.tile([128, NQ], BF16, name="h2")
        nc.scalar.activation(h2[:], u2[:, 0, :], Relu, bias=b2[:, 0:1])
        for co in range(2):
            u3 = ps_t.tile([128, 2, NQ], F32, tag="t")
            nc.tensor.matmul(u3[:, 0, :], w3t[:, co * 128:(co + 1) * 128],
                             h2[:], start=True, stop=True)
            ot = sb.tile([128, NQ], F32, name=f"ot{co}")
            if has_b3:
                tb = sb.tile([128, NQ], F32, name=f"tb{co}")
                nc.scalar.activation(tb[:], u3[:, 0, :], Id, bias=b3[:, co:co + 1])
                nc.vector.tensor_add(ot[:], tb[:], xqr[co][:])
            else:
                nc.vector.tensor_add(ot[:], u3[:, 0, :], xqr[co][:])
            nc.sync.dma_start(out_d[co * 128:(co + 1) * 128, :], ot[:])

    nc.compile()
    return nc


def _prep_inputs(inputs):
    import ml_dtypes
    bf = lambda a: np.ascontiguousarray(
        np.asarray(a, dtype=np.float32).astype(ml_dtypes.bfloat16))
    f = lambda a: np.ascontiguousarray(np.asarray(a, dtype=np.float32))
    x32 = f(inputs["corr_feat_belief"][0])                  # [D, N]
    spT = np.asarray(inputs["spatial_compatibility"][0]).T  # [N(keys), N(queries)]
    Wq, bq = f(inputs["Wq"]), f(inputs["bq"])
    Wk, bk = f(inputs["Wk"]), f(inputs["bk"])
    Wv, bv = f(inputs["Wv"]), f(inputs["bv"])
    W1, b1, g1, be1 = f(inputs["W1"]), f(inputs["b1"]), f(inputs["g1"]), f(inputs["be1"])
    W2, b2, g2, be2 = f(inputs["W2"]), f(inputs["b2"]), f(inputs["g2"]), f(inputs["be2"])
    W3, b3 = f(inputs["W3"]), f(inputs["b3"])

    scale = np.float32(1.0 / np.sqrt(DH))
    s1 = (g1 / np.sqrt(np.float32(1.0) + np.float32(1e-5))).astype(np.float32)
    s2 = (g2 / np.sqrt(np.float32(1.0) + np.float32(1e-5))).astype(np.float32)

    spT_bf = bf(spT)
    x_bf = bf(x32)
    common = dict(
        x=x_bf,
        wqt=bf(Wq.T * scale),
        wkt=bf(Wk.T),
        wvt=bf(Wv.T),
        w1t=bf((W1 * s1[:, None]).T),
        w2t=bf((W2 * s2[:, None]).T),
        w3t=bf(W3.T),
        bq2=f((bq * scale).reshape(2, 128).T),
        bk2=f(bk.reshape(2, 128).T),
        bv2=f(bv.reshape(2, 128).T),
        b1f=f((s1 * b1 + be1).reshape(128, 1)),
        b2f=f((s2 * b2 + be2).reshape(128, 1)),
        b32=f(b3.reshape(2, 128).T),
    )
    in_maps = []
    for m in range(NCORES):
        sl = slice(m * NQ, (m + 1) * NQ)
        im = dict(common)
        im["xq"] = np.ascontiguousarray(x_bf[:, sl])
        im["xqr"] = f(x32[:, sl])
        im["spt"] = np.ascontiguousarray(spT_bf[:, sl])
        in_maps.append(im)
    flags = tuple(bool(np.any(b != 0)) for b in (bq, bk, bv, b3))
    return in_maps, flags


def _run(inputs, trace=False):
    from concourse.bass_utils import run_bass_kernel_spmd
    in_maps, flags = _prep_inputs(inputs)
    if flags not in _CACHE:
        _CACHE[flags] = _build(*flags)
    nc = _CACHE[flags]
    res = run_bass_kernel_spmd(nc, in_maps, core_ids=list(range(NCORES)),
                               trace=trace)
    out = np.concatenate([res.results[m]["out"] for m in range(NCORES)],
                         axis=1)[None]
    return np.ascontiguousarray(out.astype(np.float32)), res


def kernel(**inputs):
    out, _ = _run(inputs, trace=False)
    return out
